# revision 2
# baseline (speedup 1.0000x reference)
"""Trainium2 Bass kernel for nn_NodeModel (GNN scatter-mean + node MLP).

Self-contained: takes FULL inputs as numpy arrays, shards by destination-node
range across 8 NeuronCores, runs a Bass/Tile kernel per core via
run_bass_kernel_spmd, and reassembles the full [500000, 8] output.

Strategy: nodes are sharded by destination range (62500 per core, no
collectives). The host sorts edges by destination and lays the per-edge
message [x[row] | edge_attr | 1] into a padded per-node slot stream
[node][17ch][G slots] in bf16. Each core dense-streams its slab, reduces over
the slot axis on the vector engine (f32 accumulation) to get per-node
sums+counts, then computes mean + the 2-layer MLP node-major on the vector
engine (u is folded into an effective bias on the host).

Layout: per-core padded node count NPAD = 128*NPP; node n -> partition n%128,
free column n//128.
"""
from contextlib import ExitStack

import numpy as np

import concourse.bacc as bacc
import concourse.mybir as mybir
import concourse.tile as tile
from concourse.bass_utils import run_bass_kernel_spmd

F_X = 8
F_E = 8
NCH = F_X + F_E + 1  # 17: x | attr | ones
H = 25
N_CORES = 8
N_NODES = 500000


def build_kernel(npp, G, chunk=16, mlp_split=3, repeat=1):
    assert G % 8 == 0
    npad = 128 * npp
    nc = bacc.Bacc("TRN2", target_bir_lowering=False)

    streamP = nc.dram_tensor("streamP", [npad, NCH, G], mybir.dt.bfloat16,
                             kind="ExternalInput")
    xownT = nc.dram_tensor("xownT", [128, npp, F_X], mybir.dt.float32,
                           kind="ExternalInput")
    w1b = nc.dram_tensor("w1b", [128, H - 1, H], mybir.dt.float32,
                         kind="ExternalInput")
    b1b = nc.dram_tensor("b1b", [128, H], mybir.dt.float32, kind="ExternalInput")
    w2b = nc.dram_tensor("w2b", [128, H, F_X], mybir.dt.float32,
                         kind="ExternalInput")
    b2b = nc.dram_tensor("b2b", [128, F_X], mybir.dt.float32,
                         kind="ExternalInput")
    out = nc.dram_tensor("out", [128, npp, F_X], mybir.dt.float32,
                         kind="ExternalOutput")

    chunks = []
    s = 0
    while s < npp:
        c = min(chunk, npp - s)
        chunks.append((s, c))
        s += c

    with tile.TileContext(nc) as tc, ExitStack() as ctx:
        const = ctx.enter_context(tc.tile_pool(name="const", bufs=1))
        accp = ctx.enter_context(tc.tile_pool(name="accum", bufs=1))
        sp = ctx.enter_context(tc.tile_pool(name="stream", bufs=3))
        mlpp = ctx.enter_context(tc.tile_pool(name="mlp", bufs=1))

        accum = accp.tile([128, npp, NCH], mybir.dt.float32)

        w1t = const.tile([128, H - 1, H], mybir.dt.float32)
        nc.sync.dma_start(out=w1t[:], in_=w1b[:])
        b1t = const.tile([128, H], mybir.dt.float32)
        nc.sync.dma_start(out=b1t[:], in_=b1b[:])
        w2t = const.tile([128, H, F_X], mybir.dt.float32)
        nc.sync.dma_start(out=w2t[:], in_=w2b[:])
        b2t = const.tile([128, F_X], mybir.dt.float32)
        nc.sync.dma_start(out=b2t[:], in_=b2b[:])
        xo = const.tile([128, npp, F_X], mybir.dt.float32)
        nc.sync.dma_start(out=xo[:], in_=xownT[:])
        outt = const.tile([128, npp, F_X], mybir.dt.float32)
        inv = const.tile([128, npp], mybir.dt.float32)

        for _ in range(repeat):
            for (s0, cs) in chunks:
                st = sp.tile([128, chunk, NCH, G], mybir.dt.bfloat16, tag="st")
                nc.sync.dma_start(
                    out=st[:, :cs, :, :].rearrange("p c f g -> p c (f g)"),
                    in_=streamP.ap().rearrange("(s p) c g -> p s (c g)",
                                               p=128)[:, s0:s0 + cs, :],
                )
                nc.vector.reduce_sum(
                    out=accum[:, s0:s0 + cs, :],
                    in_=st[:, :cs, :, :],
                    axis=mybir.AxisListType.X,
                )

            # ---- mean + MLP (node-major on vector engine) ----
            nc.vector.tensor_scalar_max(out=inv[:], in0=accum[:, :, NCH - 1],
                                        scalar1=1.0)
            nc.vector.reciprocal(out=inv[:], in_=inv[:])

            msz = (npp + mlp_split - 1) // mlp_split
            for m0 in range(0, npp, msz):
                mc = min(msz, npp - m0)
                feat = mlpp.tile([128, msz, H - 1], mybir.dt.float32, tag="feat")
                nc.vector.tensor_copy(out=feat[:, :mc, 0:F_X],
                                      in_=xo[:, m0:m0 + mc, :])
                for ci in range(F_X + F_E):
                    nc.vector.tensor_tensor(
                        out=feat[:, :mc, F_X + ci],
                        in0=accum[:, m0:m0 + mc, ci],
                        in1=inv[:, m0:m0 + mc],
                        op=mybir.AluOpType.mult,
                    )
                h = mlpp.tile([128, msz, H], mybir.dt.float32, tag="h")
                for j in range(H):
                    nc.vector.scalar_tensor_tensor(
                        out=h[:, :mc, j],
                        in0=feat[:, :mc, 0],
                        scalar=w1t[:, 0, j:j + 1],
                        in1=b1t[:, j:j + 1].to_broadcast([128, mc]),
                        op0=mybir.AluOpType.mult,
                        op1=mybir.AluOpType.add,
                    )
                    for k in range(1, H - 1):
                        nc.vector.scalar_tensor_tensor(
                            out=h[:, :mc, j],
                            in0=feat[:, :mc, k],
                            scalar=w1t[:, k, j:j + 1],
                            in1=h[:, :mc, j],
                            op0=mybir.AluOpType.mult,
                            op1=mybir.AluOpType.add,
                        )
                nc.vector.tensor_scalar_max(out=h[:, :mc, :], in0=h[:, :mc, :],
                                            scalar1=0.0)
                for cch in range(F_X):
                    nc.vector.scalar_tensor_tensor(
                        out=outt[:, m0:m0 + mc, cch],
                        in0=h[:, :mc, 0],
                        scalar=w2t[:, 0, cch:cch + 1],
                        in1=b2t[:, cch:cch + 1].to_broadcast([128, mc]),
                        op0=mybir.AluOpType.mult,
                        op1=mybir.AluOpType.add,
                    )
                    for j in range(1, H):
                        nc.vector.scalar_tensor_tensor(
                            out=outt[:, m0:m0 + mc, cch],
                            in0=h[:, :mc, j],
                            scalar=w2t[:, j, cch:cch + 1],
                            in1=outt[:, m0:m0 + mc, cch],
                            op0=mybir.AluOpType.mult,
                            op1=mybir.AluOpType.add,
                        )
        nc.sync.dma_start(out=out[:], in_=outt[:])

    nc.compile()
    return nc


def _to_bf16_bytes(a_f32):
    """f32 -> bf16 (round-to-nearest-even) as uint16 view."""
    u = a_f32.view(np.uint32)
    rounded = (u + 0x7FFF + ((u >> 16) & 1)) >> 16
    return rounded.astype(np.uint16)


def prep_core_inputs(x, row, col, edge_attr, W1, b1, W2, b2, u,
                     n_nodes=N_NODES, n_cores=N_CORES, G=None):
    n_per = n_nodes // n_cores
    deg = np.bincount(col, minlength=n_nodes)
    maxdeg = int(deg.max()) if len(col) else 0
    if G is None:
        G = max(8, (maxdeg + 7) // 8 * 8)
    assert maxdeg <= G, (maxdeg, G)

    order = np.argsort(col, kind="stable")
    sc = col[order]
    within = np.arange(len(col), dtype=np.int64) - \
        np.concatenate([[0], np.cumsum(deg)[:-1]])[sc]

    # message = [x[row] | attr | 1] per edge, bf16
    msg = np.empty((len(col), NCH), np.float32)
    msg[:, :F_X] = x[row[order]]
    msg[:, F_X:F_X + F_E] = edge_attr[order]
    msg[:, NCH - 1] = 1.0
    msg16 = _to_bf16_bytes(msg)

    stream = np.zeros((n_nodes, NCH, G), np.uint16)
    stream[sc, :, within] = msg16

    b1_eff = (b1 + u[0] * W1[H - 1]).astype(np.float32)
    w1b = np.ascontiguousarray(np.broadcast_to(W1[:H - 1], (128, H - 1, H)),
                               np.float32)
    b1b = np.ascontiguousarray(np.broadcast_to(b1_eff, (128, H)), np.float32)
    w2b = np.ascontiguousarray(np.broadcast_to(W2, (128, H, F_X)), np.float32)
    b2b = np.ascontiguousarray(np.broadcast_to(b2, (128, F_X)), np.float32)

    npp = (n_per + 127) // 128
    npad = npp * 128

    in_maps = []
    for c in range(n_cores):
        lo = c * n_per
        hi = lo + n_per
        s_c = np.zeros((npad, NCH, G), np.uint16)
        s_c[:n_per] = stream[lo:hi]
        xo_c = np.zeros((npad, F_X), np.float32)
        xo_c[:n_per] = x[lo:hi]
        in_maps.append({
            "streamP": s_c.view(np.dtype("uint16")),
            "xownT": np.ascontiguousarray(
                xo_c.reshape(npp, 128, F_X).transpose(1, 0, 2)),
            "w1b": w1b, "b1b": b1b, "w2b": w2b, "b2b": b2b,
        })
    meta = dict(G=G, npp=npp, npad=npad, n_per=n_per)
    return in_maps, meta


def assemble_output(results, meta, n_nodes=N_NODES, n_cores=N_CORES):
    n_per = n_nodes // n_cores
    parts = []
    for c in range(n_cores):
        o = results[c]["out"]
        o = o.transpose(1, 0, 2).reshape(meta["npad"], F_X)[:n_per]
        parts.append(o)
    return np.concatenate(parts, 0)


LAST_RUN = {}


def kernel(x, edge_index, edge_attr, u, batch, W1, b1, W2, b2):
    x = np.asarray(x, np.float32)
    edge_attr = np.asarray(edge_attr, np.float32)
    u = np.asarray(u, np.float32)
    W1 = np.asarray(W1, np.float32)
    b1 = np.asarray(b1, np.float32)
    W2 = np.asarray(W2, np.float32)
    b2 = np.asarray(b2, np.float32)
    row = np.asarray(edge_index[0]).astype(np.int32)
    col = np.asarray(edge_index[1]).astype(np.int32)

    in_maps, meta = prep_core_inputs(x, row, col, edge_attr, W1, b1, W2, b2, u)
    nc = build_kernel(meta["npp"], meta["G"])
    # bf16 tensors are passed as uint16; bass expects ml_dtypes bfloat16 view
    import ml_dtypes
    for m in in_maps:
        m["streamP"] = m["streamP"].view(ml_dtypes.bfloat16)
    res = run_bass_kernel_spmd(nc, in_maps, core_ids=list(range(N_CORES)))
    LAST_RUN.update(nc=nc, in_maps=in_maps, meta=meta)
    return assemble_output(res.results, meta).astype(np.float32)
